# revision 70
# baseline (speedup 1.0000x reference)
"""DeBERTa-v3 disentangled self-attention on 8 TRN2 NeuronCores — v2.

Sharding: batch (2-way) x head-groups (4-way of 3 heads) = 8 cores.

v2 exploits bucket saturation: f_c(d)=511 for d>=506 and 0 for d<=-512,
and g(j-i) == f_c(i-j) exactly. Blocks with |it-jt|>=5 are fully
saturated: c2p+p2c there is rank-2 (s[i] + u[j]); s is added into the
score psum as a K=1 matmul, u via the ACT exp's per-partition bias.
Only the |it-jt|<=4 near band (|delta|<=639) needs the exact window:
a single shared 1280-column PKe/PQe expansion (columns are RELATIVE
deltas, identical for every tile), H strips via fp32r matmuls, the
Toeplitz shift via skewed flat-AP SBUF->SBUF DMA, c2p transposed into
the score psum by PE-ident matmuls, p2c added by PE-ident matmuls.

Matmul dtypes: fp32r (1 cyc/row at ap>=256; inputs must be produced by
an f32r-writing instruction — DVE copies / f32r-declared dram loads;
memsets must go through a .bitcast(F32) view) for projections, H strips
and qk; bf16 for c2p strips/transposes, p2c strips/ident-adds, AV and
out-proj. Score layout S_T = [key j, query i].

v3: the saturated rank-1 s-row adds are folded into the qk matmul via
K-slicing: qT64 rows 0/32 carry srowL and srowR-srowL (computed by one
pkq3 matmul per 512-chunk), kTe = [ones rows 0/32; zeros; kT] so slice
[0:128] adds srowR to saturated-R cols, [64:128] is the plain band qk,
and a base-0 rank-1 adds srowL on the L tail. c2p_nat uses a ragged
per-tile layout (_OFFS). The c2p shift DMA is lagged one prelude step
and hct is 4-deep so the SP DMA queue never stalls the chain; H_p
strips run two key tiles ahead (first two emitted from the prelude);
the denominator + out-proj are interleaved per 512-chunk into the
jt=15 loop. h0's mk_hc(6..15) are deferred into its own jt loop (one
per jt at ic=1), shrinking the serial head-0 prelude; kTe constant
rows are memset on the idle Pool engine via an F32 bitcast view; the
v projection runs fully in f32r against a 256-padded wvb (no separate
bf16 hidden_states load, and v gains f32 input precision). On the
last head the Wo01 halves of the out-proj prestart on spsum while the
DVE normalize chain runs (rbc uses the then-idle hpsum pool), and fin
is 4-deep so output-DMA backlog on the SP queue doesn't stall the
fps->fo->DMA chain.

Scheduling: next head's prelude (qT64/kTe relocate, gather, H_c strips,
s/u rows, first H_p strips) is software-pipelined one step per score
chunk into the current head's key-tile loop; the AV matmul is delayed
one 512-chunk so PE never waits on the exp.
HW-validated: rel err ~9.6e-3 vs the f32 reference (gate 2e-2);
cost-model timeline ~326us/core (v2: 374us; f32 baseline: 919us).
"""

import math
import sys

sys.path.insert(0, "/opt/trn_rl_repo")

import numpy as np
import ml_dtypes

import concourse.bass as bass
import concourse.bacc as bacc
import concourse.tile as tile
from concourse import mybir
from concourse.bass_utils import run_bass_kernel_spmd

B, S, HID, H = 2, 2048, 768, 12
D = HID // H                    # 64
NH = 3                          # heads per core
BUCKET = 256
MAX_POS = 512
SPAN = BUCKET                   # 256
SCALE = 1.0 / math.sqrt(D)      # 0.125
MID = BUCKET // 2               # 128
NB = 2 * SPAN                   # 512 buckets

NT = S // 128                   # 16 tiles of 128
WC = 1280                       # shared relative-delta window width
D0 = 640                        # PKe col c <-> delta D0-c ; PQe col c <-> c-D0
F32 = mybir.dt.float32
F32R = mybir.dt.float32r
BF16 = mybir.dt.bfloat16
FP8 = mybir.dt.float8e4
DR = mybir.MatmulPerfMode.DoubleRow
I16 = mybir.dt.int16

_CACHE = {}
TRACE = False      # set by test.py to capture an NTFF profile
LAST = None        # BassKernelResults of the most recent run


def _lo(t):
    return max(0, 128 * (t - 4))


def _hi(t):
    return min(S, 128 * (t + 5))


_OFFS = []
_tot = 0
for _t in range(16):
    _OFFS.append(_tot)
    _tot += _hi(_t) - _lo(_t)
C2P_COLS = _tot


def _chunks_for(t):
    """Which 512-col chunks of the WC window tile t's shift actually reads."""
    off = D0 - 128 * t + _lo(t)
    cmin = off - 127
    cmax = off + (_hi(t) - _lo(t)) - 1
    out = []
    for c0, c1 in ((0, 512), (512, 1024), (1024, 1280)):
        if c1 > cmin and c0 <= cmax:
            out.append((c0, c1))
    return out


def _log_bucket(rel_pos):
    """numpy float32 replica of the reference jax _log_bucket."""
    rp = rel_pos.astype(np.float32)
    abs_pos = np.where(
        (rel_pos < MID) & (rel_pos > -MID), np.float32(MID - 1), np.abs(rp)
    )
    log_pos = (
        np.ceil(
            np.log(abs_pos / np.float32(MID))
            * np.float32(MID - 1)
            / np.float32(math.log((MAX_POS - 1) / MID))
        )
        + np.float32(MID)
    )
    bucket = np.where(abs_pos <= MID, rp, log_pos * np.sign(rp))
    return bucket.astype(np.int32)


def _gather_idx():
    """Wrapped int16 idx tensor [128, WC//16] for the PKQe ap_gather.

    Column c of PKe (partitions 0-63, Q7 cores 0-3) holds bucket
    f_c(D0 - c); PQe (partitions 64-127) holds f_c(c - D0).
    """
    m = np.arange(WC)
    mc = np.clip(_log_bucket(D0 - m) + SPAN, 0, NB - 1).astype(np.int16)
    mp = np.clip(_log_bucket(m - D0) + SPAN, 0, NB - 1).astype(np.int16)
    gi = np.zeros((128, WC // 16), dtype=np.int16)
    for c in range(8):
        vals = mc if c < 4 else mp
        for mm in range(WC):
            gi[16 * c + mm % 16, mm // 16] = vals[mm]
    return gi


KC_ = HID // 128


def build_kernel():
    nc = bacc.Bacc("TRN2", target_bir_lowering=False, debug=False)

    hst = nc.dram_tensor("hst", [HID, S], F32R, kind="ExternalInput")
    relt = nc.dram_tensor("relt", [HID, NB], F32R, kind="ExternalInput")
    wqk = nc.dram_tensor("wqk", [128, NH, KC_, 128], F32R, kind="ExternalInput")
    wvb = nc.dram_tensor("wvb", [128, KC_, 256], F32R, kind="ExternalInput")
    wpos = nc.dram_tensor("wpos", [128, NH, KC_, 128], F32R, kind="ExternalInput")
    wo = nc.dram_tensor("wo", [NH, D, HID], BF16, kind="ExternalInput")
    ident_d = nc.dram_tensor("ident", [128, 128], BF16, kind="ExternalInput")
    gidx_d = nc.dram_tensor("gidx", [128, WC // 16], I16, kind="ExternalInput")
    outt = nc.dram_tensor("outt", [HID, S], F32, kind="ExternalOutput")

    KC = HID // 128             # 6 contraction chunks

    with tile.TileContext(nc) as tc:
        with (
            tc.tile_pool(name="persist", bufs=1) as persist,
            tc.tile_pool(name="heads", bufs=1) as heads,
        ):
            ident = persist.tile([128, 128], BF16)
            nc.sync.dma_start(ident[:], ident_d[:])
            gidx = persist.tile([128, WC // 16], I16)
            nc.sync.dma_start(gidx[:], gidx_d[:])
            ones64f = persist.tile([128, 64], F32)
            nc.vector.memset(ones64f[:], 1.0)
            ones64r = persist.tile([128, 64], F32R)
            nc.vector.tensor_copy(ones64r[:], ones64f[:])

            # per-head persistent tiles
            qkT = [heads.tile([128, S], F32R, tag=f"qkT{h}", name=f"qkT{h}") for h in range(NH)]
            v_sb = [heads.tile([128, NT, 65], BF16, tag=f"v{h}", name=f"v{h}") for h in range(NH)]
            pkqT = [heads.tile([128, NB], F32R, tag=f"pkqT{h}", name=f"pkqT{h}") for h in range(NH)]
            oT2 = heads.tile([128, S], BF16, tag="oT2", name="oT2")
            oT1 = heads.tile([64, S], BF16, tag="oT1", name="oT1")
            oT_sc = [oT2[0:64, :], oT2[64:128, :], oT1[0:64, :]]
            wo01_sb = heads.tile([128, HID], BF16, tag="wo01", name="wo01")
            wo2_sb = heads.tile([64, HID], BF16, tag="wo2", name="wo2")
            # exp biases per key tile (side 0 = L, side 1 = R)
            u_cols = [heads.tile([128, NT, 2], F32, tag=f"uc{h}", name=f"uc{h}") for h in range(NH)]

            # ---------------- projections ----------------
            with (
                tc.tile_pool(name="prep", bufs=1) as prep,
                tc.tile_pool(name="wpool", bufs=1) as wpool,
                tc.tile_pool(name="ppsum", bufs=2, space="PSUM") as ppsum,
                tc.tile_pool(name="vpsum", bufs=2, space="PSUM") as vpsum,
            ):
                # DMA order tracks first use: head-0 weights, then hsT
                # n0 chunks (first matmul group), then the rest
                wqk_sb = wpool.tile([128, NH, KC, 128], F32R)
                wvb_sb = wpool.tile([128, KC, 256], F32R)
                wpos_sb = wpool.tile([128, NH, KC, 128], F32R)
                hsT = prep.tile([128, KC, S], F32R)
                relT = prep.tile([128, KC, NB], F32R)
                hst_r = hst.rearrange("(k p) n -> p k n", p=128)
                relt_r = relt.rearrange("(k p) n -> p k n", p=128)
                nc.sync.dma_start(wqk_sb[:, 0, :, :], wqk[:, 0, :, :])
                for k in range(KC):
                    nc.sync.dma_start(
                        hsT[:, k, 0:512], hst_r[:, k, 0:512]
                    )
                for k in range(KC):
                    nc.sync.dma_start(
                        hsT[:, k, bass.ts(1, 512)],
                        hst_r[:, k, bass.ts(1, 512)],
                    )
                nc.sync.dma_start(wqk_sb[:, 1, :, :], wqk[:, 1, :, :])
                for k in range(KC):
                    nc.sync.dma_start(
                        hsT[:, k, bass.ts(2, 512)],
                        hst_r[:, k, bass.ts(2, 512)],
                    )
                nc.sync.dma_start(wqk_sb[:, 2, :, :], wqk[:, 2, :, :])
                for k in range(KC):
                    nc.sync.dma_start(
                        hsT[:, k, bass.ts(3, 512)],
                        hst_r[:, k, bass.ts(3, 512)],
                    )
                nc.sync.dma_start(wpos_sb[:], wpos[:])
                for k in range(KC):
                    nc.sync.dma_start(relT[:, k, :], relt_r[:, k, :])
                nc.sync.dma_start(wvb_sb[:], wvb[:])
                nc.sync.dma_start(
                    wo01_sb[:], wo.rearrange("h d m -> (h d) m")[0:128, :]
                )
                nc.sync.dma_start(wo2_sb[:], wo[2, :, :])

                # q/k projections: psum rows 0-63 = qT, 64-127 = kT
                for h in range(NH):
                    for n in range(4):
                        ps = ppsum.tile([128, 512], F32, tag="pp")
                        for k in range(KC):
                            nc.tensor.matmul(
                                ps[:],
                                wqk_sb[:, h, k, :],
                                hsT[:, k, bass.ts(n, 512)],
                                start=(k == 0),
                                stop=(k == KC - 1),
                            )
                        nc.vector.tensor_copy(
                            qkT[h][:, bass.ts(n, 512)], ps[:]
                        )
                    # pos-key/query projections (scaled by 1/SCALE on host):
                    # psum rows 0-63 = PKT, 64-127 = PQT
                    ps2 = ppsum.tile([128, NB], F32, tag="pp2")
                    for k in range(KC):
                        nc.tensor.matmul(
                            ps2[:],
                            wpos_sb[:, h, k, :],
                            relT[:, k, :],
                            start=(k == 0),
                            stop=(k == KC - 1),
                        )
                    nc.vector.tensor_copy(pkqT[h][:], ps2[:])

                # v projection per key tile (bf16)
                for t in range(NT):
                    psv = vpsum.tile([128, 256], F32, tag="pv")
                    for k in range(KC):
                        nc.tensor.matmul(
                            psv[:],
                            hsT[:, k, bass.ts(t, 128)],
                            wvb_sb[:, k, :],
                            start=(k == 0),
                            stop=(k == KC - 1),
                        )
                    for h in range(NH):
                        nc.scalar.copy(
                            v_sb[h][:, t, 0:64], psv[:, bass.ts(h, D)]
                        )
                        nc.vector.memset(v_sb[h][:, t, 64:65], 1.0)

            # ---------------- attention ----------------
            with (
                tc.tile_pool(name="hc", bufs=2) as hcpool,
                tc.tile_pool(name="srows", bufs=1) as srows_p,
                tc.tile_pool(name="strips", bufs=4) as strips,
                tc.tile_pool(name="att", bufs=2) as att,
                tc.tile_pool(name="hctp", bufs=4) as hctp,
                tc.tile_pool(name="hpstage", bufs=2) as hpstage,
                tc.tile_pool(name="pchunks", bufs=3) as pchunks,
                tc.tile_pool(name="oacc", bufs=1) as oacc,
                tc.tile_pool(name="fin", bufs=4) as fin,
                tc.tile_pool(name="hpsum", bufs=2, space="PSUM") as hpsum,
                tc.tile_pool(name="spsum", bufs=2, space="PSUM") as spsum,
                tc.tile_pool(name="opsum", bufs=1, space="PSUM") as opsum,
            ):
                def emit_hp(sth, hh, jt, eng=nc.sync):
                    j0 = jt * 128
                    ilo = _lo(jt)
                    w_jt = _hi(jt) - ilo
                    hp_sb = hpstage.tile([128, WC], BF16, tag="hp",
                                         name="hp_sb")
                    for c0, c1 in _chunks_for(jt):
                        hps = hpsum.tile([128, 512], F32, tag="hps",
                                         name="hps")
                        nc.tensor.matmul(
                            hps[:, 0:c1 - c0],
                            qkT[hh][64:128, bass.ts(jt, 128)],
                            sth["pkqe"][64:128, c0:c1],
                            start=True, stop=True,
                        )
                        nc.vector.tensor_copy(
                            hp_sb[:, c0:c1], hps[:, 0:c1 - c0]
                        )
                    p2c = strips.tile([128, 1152], BF16, tag="p2c",
                                      name="p2c")
                    eng.dma_start(
                        p2c[:, 0:w_jt],
                        bass.AP(
                            tensor=hp_sb.tensor,
                            offset=hp_sb.offset + D0 - j0 + ilo,
                            ap=[[WC - 1, 128], [1, w_jt]],
                        ),
                    )
                    sth["strips"][jt] = p2c

                def make_prelude(h):
                    st = {"hct": {}}

                    def hc_shift(t):
                        # skew DMA for tile t, lagged one prelude step so
                        # the SP queue's copy-done wait is already satisfied.
                        # h0's upfront shifts (t<5) predate all exp work, so
                        # they alternate onto the idle ACT HWDGE queue.
                        hc_t = st["hct"].pop(t)
                        w_t = _hi(t) - _lo(t)
                        nc.sync.dma_start(
                            st["c2p"][:, _OFFS[t]:_OFFS[t] + w_t],
                            bass.AP(
                                tensor=hc_t.tensor,
                                offset=hc_t.offset + D0 - 128 * t + _lo(t),
                                ap=[[WC - 1, 128], [1, w_t]],
                            ),
                        )

                    def s0():
                        # qT64 rows 64:128 = qT; rows 62/63 get the srow
                        # fold rows (s_su). kTe rows 64:128 = kT with ones
                        # rows 62/63 above: K-sliced qk matmuls add the
                        # saturated rank-1 terms for free.
                        st["qT64"] = hcpool.tile(
                            [128, S], F32R, tag="qT64", name="qT64"
                        )
                        nc.sync.dma_start(
                            st["qT64"][64:128, :], qkT[h][0:64, :]
                        )
                        st["kTe"] = hcpool.tile(
                            [128, S], F32R, tag="kTe", name="kTe"
                        )
                        nc.sync.dma_start(
                            st["kTe"][64:128, :], qkT[h][64:128, :]
                        )
                        nc.gpsimd.memset(
                            st["kTe"][0:64, :].bitcast(F32), 0.0
                        )
                        nc.gpsimd.memset(
                            st["kTe"][0:1, :].bitcast(F32), 1.0
                        )
                        nc.gpsimd.memset(
                            st["kTe"][32:33, :].bitcast(F32), 1.0
                        )
                        # gpsimd gather ucode can't handle the f32r dtype
                        # tag: gather plain f32, round via a DVE copy
                        pkqe_raw = att.tile([128, WC], F32, tag="hp", name="pkqeraw")
                        nc.gpsimd.ap_gather(
                            out_ap=pkqe_raw[:],
                            in_ap=pkqT[h][:].bitcast(F32),
                            idxs_ap=gidx[:],
                            channels=128, num_elems=NB, d=1, num_idxs=WC,
                        )
                        st["pkqe"] = hcpool.tile([128, WC], F32R, tag="pkqe", name="pkqe")
                        nc.vector.tensor_copy(st["pkqe"][:], pkqe_raw[:])

                    def mk_hc(t):
                        def s():
                            if "c2p" not in st:
                                st["c2p"] = hcpool.tile(
                                    [128, C2P_COLS], BF16, tag="c2pn",
                                    name="c2pn",
                                )
                            pkqe = st["pkqe"]
                            hc_t = hctp.tile([128, WC], BF16, tag="hct")
                            for c0, c1 in _chunks_for(t):
                                hps = hpsum.tile([128, 512], F32, tag="hps")
                                nc.tensor.matmul(
                                    hps[:, 0:c1 - c0],
                                    qkT[h][0:64, bass.ts(t, 128)],
                                    pkqe[0:64, c0:c1],
                                    start=True, stop=True,
                                )
                                if (t + c0 // 512) % 2 == 0:
                                    nc.scalar.copy(
                                        hc_t[:, c0:c1],
                                        hps[:, 0:c1 - c0],
                                    )
                                else:
                                    nc.vector.tensor_copy(
                                        hc_t[:, c0:c1],
                                        hps[:, 0:c1 - c0],
                                    )
                            st["hct"][t] = hc_t
                            if t > 0:
                                hc_shift(t - 1)
                        return s

                    def s_su():
                        hc_shift(NT - 1)
                        # srow fold rows into qT64 rows 0:64: row 0 =
                        # srowR - srowL = q.(pk0 - pk511), row 32 = srowL =
                        # q.pk511, other rows zero. kTe has ones at rows
                        # 0/32 and zeros elsewhere in 0:64, so K-slice
                        # [0:128] adds srowR, [32:128] adds srowL, [64:128]
                        # is the plain band qk.
                        pkq3f = srows_p.tile([64, 64], F32, tag="pkq3f")
                        nc.vector.memset(pkq3f[:], 0.0)
                        nc.vector.tensor_copy(
                            pkq3f[:, 0:1], pkqT[h][0:64, 511:512]
                        )
                        nc.vector.tensor_sub(
                            pkq3f[:, 32:33], pkqT[h][0:64, 0:1],
                            pkqT[h][0:64, 511:512],
                        )
                        pkq3 = srows_p.tile([64, 64], F32R, tag="pkq3")
                        nc.vector.tensor_copy(pkq3[:], pkq3f[:])
                        for n in range(4):
                            sl = bass.ts(n, 512)
                            spA = hpsum.tile([128, 512], F32, tag="hps")
                            nc.tensor.matmul(
                                spA[0:64, :], pkq3[:, :],
                                qkT[h][0:64, sl],
                                start=True, stop=True, skip_group_check=True,
                            )
                            nc.vector.tensor_copy(
                                st["qT64"][0:64, sl], spA[0:64, :]
                            )
                        # u biases: [L, R] column pair, one 2-wide mm per jt
                        pkq2 = srows_p.tile([128, 2], F32R, tag="pkq2")
                        nc.vector.tensor_copy(pkq2[:, 0:1], pkqT[h][:, 511:512])
                        nc.vector.tensor_copy(pkq2[:, 1:2], pkqT[h][:, 0:1])
                        ups = hpsum.tile([128, 512], F32, tag="hps")
                        for jt in range(NT):
                            nc.tensor.matmul(
                                ups[:, 2 * jt:2 * jt + 2],
                                qkT[h][64:128, bass.ts(jt, 128)],
                                pkq2[64:128, 0:2],
                                start=True, stop=True, skip_group_check=True,
                            )
                        nc.scalar.mul(
                            u_cols[h][:, :, :], ups[:, 0:2 * NT], SCALE
                        )

                    def mk_hp(jt):
                        def s():
                            emit_hp(st, h, jt)
                        return s

                    st["strips"] = {}
                    return st, ([s0] + [mk_hc(t) for t in range(NT)]
                                + [s_su] + [mk_hp(t) for t in range(2)])

                cur, steps = make_prelude(0)
                for sfn in steps:
                    sfn()
                for h in range(NH):
                    qT64 = cur["qT64"]
                    kTe = cur["kTe"]
                    pkqe = cur["pkqe"]
                    c2p_nat = cur["c2p"]
                    if h + 1 < NH:
                        nxt, nsteps = make_prelude(h + 1)

                    oT_ps = opsum.tile([65, S], F32, tag="ot")
                    oT_acc = oacc.tile([65, S], F32R, tag="otacc")

                    # H_p + shift prefetched three key tiles ahead; AV
                    # delayed one chunk so PE never waits on the exp
                    strip_tiles = cur["strips"]

                    pending_av = [None]

                    def emit_pending():
                        if pending_av[0] is not None:
                            pending_av[0]()
                            pending_av[0] = None

                    def denom(ic, h=h, oT_ps=oT_ps, oT_acc=oT_acc):
                        sl = bass.ts(ic, 512)
                        nc.vector.tensor_copy(oT_acc[:, sl], oT_ps[:, sl])
                        with nc.allow_low_precision(
                            reason="f32r is f32-width; rounding only"
                        ):
                            nc.vector.reciprocal(
                                oT_acc[64:65, sl], oT_acc[64:65, sl]
                            )
                        last = h == NH - 1
                        # last head: rbc in the now-idle hpsum pool so the
                        # Wo01 projection halves can prestart on spsum and
                        # fill PE while the DVE normalize chain runs
                        rpool = hpsum if last else spsum
                        rtag = "hps" if last else "sps"
                        rbc = rpool.tile([128, 512], F32, tag=rtag,
                                         name="rbc")
                        fps_t = []
                        if last:
                            for m in range(2):
                                fps = spsum.tile([128, 512], F32, tag="sps",
                                                 name="fps")
                                nc.tensor.matmul(
                                    fps[:],
                                    wo01_sb[:, bass.ts(m, 128)],
                                    oT2[:, sl],
                                    start=True, stop=False,
                                    skip_group_check=True,
                                )
                                fps_t.append(fps)
                        nc.tensor.matmul(
                            rbc[0:64, 0:512],
                            ones64r[64:65, :],
                            oT_acc[64:65, sl],
                            start=True, stop=True, skip_group_check=True,
                        )
                        nc.vector.tensor_mul(
                            oT_sc[h][:, sl],
                            oT_acc[0:64, sl],
                            rbc[0:64, 0:512],
                        )
                        # last head: its oT chunk is final -> project now
                        if last:
                            for m in range(6):
                                if m < 2:
                                    fps = fps_t[m]
                                else:
                                    fps = spsum.tile([128, 512], F32,
                                                     tag="sps", name="fps")
                                    nc.tensor.matmul(
                                        fps[:],
                                        wo01_sb[:, bass.ts(m, 128)],
                                        oT2[:, sl],
                                        start=True, stop=False,
                                        skip_group_check=True,
                                    )
                                nc.tensor.matmul(
                                    fps[:],
                                    wo2_sb[:, bass.ts(m, 128)],
                                    oT1[:, sl],
                                    start=False, stop=True,
                                    skip_group_check=True,
                                )
                                fo = fin.tile([128, 512], F32, tag="fo")
                                nc.scalar.copy(fo[:], fps[:])
                                nc.sync.dma_start(
                                    outt[bass.ts(m, 128), bass.ts(ic, 512)],
                                    fo[:],
                                )

                    for jt in range(NT):
                        j0 = jt * 128
                        ilo, ihi = _lo(jt), _hi(jt)
                        if jt + 2 < NT:
                            emit_hp(cur, h, jt + 2)
                        p2c_st = strip_tiles.pop(jt)

                        for ic in range(4):
                            c0 = 512 * ic
                            c1 = c0 + 512
                            sps = spsum.tile([128, 512], F32, tag="sps")
                            a = max(c0, ilo)
                            b2 = min(c1, ihi)
                            aL = max(c0, ihi)
                            bR = min(c1, ilo)
                            ops = []
                            # qk segments: K-slice [0:128] folds +srowR into
                            # the saturated-R cols (row 0 = srowR - srowL,
                            # row 32 = srowL, ones in kTe rows 0/32); the
                            # band+L cols use the plain [64:128] slice with
                            # an explicit srowL rank-1 on the L part.
                            for ra, rb, klo in (
                                (c0, bR, 0), (a, c1, 64)
                            ):
                                if ra < rb:
                                    ops.append(
                                        lambda st_, sp_, ra=ra, rb=rb,
                                        klo=klo, sps=sps: nc.tensor.matmul(
                                            sps[:, ra - c0:rb - c0],
                                            kTe[klo:128, bass.ts(jt, 128)],
                                            qT64[klo:128, ra:rb],
                                            start=st_, stop=sp_,
                                            skip_group_check=True,
                                        ))
                            if aL < c1:
                                ops.append(lambda st_, sp_, aL=aL, sps=sps:
                                    nc.tensor.matmul(
                                        sps[:, aL - c0:512],
                                        kTe[0:1, bass.ts(jt, 128)],
                                        qT64[0:1, aL:c1],
                                        start=False, stop=sp_,
                                        skip_group_check=True,
                                    ))
                            # c2p transpose-adds for near query tiles in chunk
                            for it in range(max(0, jt - 4), min(NT, jt + 5)):
                                i0 = 128 * it
                                if not (c0 <= i0 < c1):
                                    continue
                                ops.append(lambda st_, sp_, it=it, i0=i0, sps=sps:
                                    nc.tensor.matmul(
                                        sps[:, i0 - c0:i0 - c0 + 128],
                                        c2p_nat[:, _OFFS[it] + j0 - _lo(it):_OFFS[it] + j0 - _lo(it) + 128],
                                        ident[:],
                                        start=False, stop=sp_,
                                        skip_group_check=True,
                                    ))
                            # p2c ident-add over near band in chunk
                            if a < b2:
                                ops.append(lambda st_, sp_, a=a, b2=b2, sps=sps:
                                    nc.tensor.matmul(
                                        sps[:, a - c0:b2 - c0],
                                        ident[:],
                                        p2c_st[:, a - ilo:b2 - ilo],
                                        start=False, stop=sp_,
                                        skip_group_check=True,
                                    ))
                            for oi, op in enumerate(ops):
                                op(oi == 0, oi == len(ops) - 1)
                            emit_pending()
                            if jt == 15 and ic >= 1:
                                denom(ic - 1)

                            # exp with per-region bias
                            p_ch = pchunks.tile([128, 512], BF16, tag="p",
                                                name="p")
                            p_sl = p_ch[:]
                            for ra, rb, bias in (
                                (c0, bR if c0 < bR else c0,
                                 u_cols[h][:, jt, 1:2]),
                                (max(c0, ilo), min(c1, ihi), 0.0),
                                (aL, c1, u_cols[h][:, jt, 0:1]),
                            ):
                                if ra < rb:
                                    nc.scalar.activation(
                                        p_sl[:, ra - c0:rb - c0],
                                        sps[:, ra - c0:rb - c0],
                                        mybir.ActivationFunctionType.Exp,
                                        bias=bias, scale=SCALE,
                                    )

                            if h + 1 < NH and jt >= 4:
                                si = (jt - 4) * 2 + (ic // 2)
                                if ic % 2 == 0 and si < len(nsteps):
                                    nsteps[si]()

                            def av(jt=jt, c0=c0, c1=c1, p_sl=p_sl):
                                nc.tensor.matmul(
                                    oT_ps[:, c0:c1],
                                    v_sb[h][:, jt, :],
                                    p_sl,
                                    start=(jt == 0), stop=(jt == 15),
                                    skip_group_check=True,
                                )
                            pending_av[0] = av

                        if jt == 15:
                            emit_pending()

                    denom(3)
                    if h + 1 < NH:
                        cur = nxt

    nc.compile()
    return nc


def host_prep(inputs):
    """Build the 8 per-core input maps from the full inputs."""
    hidden_states = np.asarray(inputs["hidden_states"], dtype=np.float32)
    rel_embeddings = np.asarray(inputs["rel_embeddings"], dtype=np.float32)
    Wq = np.asarray(inputs["Wq"], np.float32)
    Wk = np.asarray(inputs["Wk"], np.float32)
    Wv = np.asarray(inputs["Wv"], np.float32)
    Wo = np.asarray(inputs["Wo"], np.float32)

    if "gidx" not in _CACHE:
        _CACHE["gidx"] = _gather_idx()
        _CACHE["ident"] = np.eye(128, dtype=ml_dtypes.bfloat16)

    inv = np.float32(1.0 / SCALE)
    KC__ = HID // 128
    hs_t = [np.ascontiguousarray(hidden_states[b].T) for b in range(B)]
    relt = np.ascontiguousarray(rel_embeddings.T)
    in_maps = []
    for core in range(8):
        b = core // 4
        h0 = NH * (core % 4)
        hs = slice(h0, h0 + NH)
        # pre-laid-out [p, h, k, m]: contiguous single-run DMAs on device
        wqk_c = np.ascontiguousarray(
            np.concatenate([Wq[:, hs, :], Wk[:, hs, :]], axis=2)
            .reshape(KC__, 128, NH, 128).transpose(1, 2, 0, 3)
        )
        wpos_c = np.ascontiguousarray(
            np.concatenate([Wk[:, hs, :] * inv, Wq[:, hs, :] * inv], axis=2)
            .reshape(KC__, 128, NH, 128).transpose(1, 2, 0, 3)
        )
        wo_c = np.ascontiguousarray(
            Wo[h0 * D: (h0 + NH) * D, :].reshape(NH, D, HID)
        ).astype(ml_dtypes.bfloat16)
        in_maps.append(
            dict(
                hst=hs_t[b],
                relt=relt,
                wqk=wqk_c,
                wvb=np.ascontiguousarray(
                    np.concatenate(
                        [Wv[:, hs, :].reshape(KC__, 128, NH * D),
                         np.zeros((KC__, 128, 256 - NH * D), np.float32)],
                        axis=2,
                    ).transpose(1, 0, 2)
                ),
                wpos=wpos_c,
                wo=wo_c,
                ident=_CACHE["ident"],
                gidx=_CACHE["gidx"],
            )
        )
    return in_maps


def kernel(hidden_states, rel_embeddings, Wq, bq, Wk, bk, Wv, bv, Wo, bo):
    # biases are zero in this problem's setup; the kernel folds only bo.
    assert not np.any(bq) and not np.any(bk) and not np.any(bv)
    bo = np.asarray(bo, np.float32)

    in_maps = host_prep(
        dict(hidden_states=hidden_states, rel_embeddings=rel_embeddings,
             Wq=Wq, Wk=Wk, Wv=Wv, Wo=Wo)
    )
    if "nc" not in _CACHE:
        _CACHE["nc"] = build_kernel()
    nc = _CACHE["nc"]

    global LAST
    res = run_bass_kernel_spmd(
        nc, in_maps, core_ids=list(range(8)), trace=TRACE
    )
    LAST = res
    out = np.zeros((B, S, HID), np.float32)
    for core in range(8):
        out[core // 4] += res.results[core]["outt"].T
    out += bo
    return out

